# revision 2
# baseline (speedup 1.0000x reference)
"""AttentionWithFastKANTransform TRN2 kernel v2 (8 cores, fp8 DoubleRow).

Phase 1 (row-sharded, R=512 rows/core): per input tensor (k, q, v):
  LN stats via PE bf16 matmuls; s,t broadcast via DRAM-bounce (bf16);
  xn = clamp(x*s + t, +-3.2); basis_j = t0 * Q_j with t0 = exp(-z0^2)
  (ACT, z0 = 1.75*xn + 3.5) and Q_j = exp(3.5j*xn + j(7-j)) via
  i16-Schraudolph (DVE tensor_scalar in 4x mode) + Pool TT into fp8.
  Spline matmuls run fp8 DoubleRow (2 k-chunks/instr at 0.5 cyc/row);
  base matmuls stay bf16; both accumulate into one PSUM group with
  weights pre-scaled x512 and the epilogue ACT applying scale 1/512.
Phase 2 (head-sharded, core h = head h): S = wk^T wq in fp8 DoubleRow
  over d-halves ([32,2,*] operands); exp split between ACT (exact) and
  DVE (Schraudolph, knob K_EXPDVE); att@V in fp8 DoubleRow over k-tile
  pairs with an appended ones column producing softmax denominators.
  S/exp for qc0 are emitted mid-phase-1 so ACT/PE overlap the lg/lv
  matmuls; att@V waits for wv (a2a1b).
Phase 3: fastkan lo on the gated, head-gathered output; same machinery.
"""

import os
import numpy as np
import ml_dtypes

import concourse.bass as bass
import concourse.bacc as bacc
import concourse.tile as tile
import concourse.mybir as mybir
from concourse.bass_utils import run_bass_kernel_spmd
from concourse.masks import make_identity

AF = mybir.ActivationFunctionType
OP = mybir.AluOpType
F32 = mybir.dt.float32
BF16 = mybir.dt.bfloat16
FP8 = mybir.dt.float8e4
I16 = mybir.dt.int16
PM = mybir.MatmulPerfMode
E4M3 = ml_dtypes.float8_e4m3
BF16NP = ml_dtypes.bfloat16

NCORES = 8
B, L, IN, OUT, H, D, G = 2, 2048, 512, 512, 8, 64, 8
R = (B * L) // NCORES          # 512 rows per core
NC_IN = IN // 128              # 4 input-dim chunks
NM = OUT // 128                # 4 output m-tiles
NKT = L // 128                 # 16 k-tiles per batch
QC = 1024                      # phase-2 q-chunk
NQC = L // QC
SCALE = 512.0                  # fp8 weight scale (epilogue undoes)
EPS = 1e-5
XCLAMP = 3.2
A16 = 128.0 / np.log(2.0)      # bf16 Schraudolph slope
B16 = 127.0 * 128.0 - 7.0      # bf16 Schraudolph bias (calibrated)
A8S = 8.0 / np.log(2.0)        # fp8e4 Schraudolph slope
B8S = 7.0 * 8.0 - 0.45         # fp8e4 Schraudolph bias
LAYERS = ("lq", "lg", "lk", "lv", "lo")

# tuning knobs
EXPDVE0 = int(os.environ.get("K_EXPDVE0", "8"))   # of 16 exp units to DVE (qc0)
EXPDVE1 = int(os.environ.get("K_EXPDVE1", "4"))   # of 16 exp units to DVE (qc1)
def _parse_tt(s):
    d = {}
    for part in s.split(","):
        if part:
            j, e = part.split(":")
            d[int(j)] = e
    return d

TT_DVE = _parse_tt(os.environ.get("K_TTDVE", "1:A,7:A,2:D,4:D,6:D"))
TT_DVE3 = _parse_tt(os.environ.get("K_TTDVE3", "1:A,7:A,2:D,3:D,4:D,6:D"))

_cache = {}


def _bcast(nc, pools, dram_pool, src_sb, n, nparts, tag):
    """Broadcast SBUF [1, n] -> SBUF [nparts, n] via a DRAM bounce."""
    bounce = dram_pool.tile([1, n], src_sb.dtype, tag=f"bounce_{tag}")
    nc.scalar.dma_start(bounce, src_sb)
    dst = pools["bc"].tile([nparts, n], src_sb.dtype, tag=f"bc_{tag}",
                           bufs=(1 if nparts < 128 else 3))
    src = bass.AP(tensor=bounce.tensor, offset=bounce.offset,
                  ap=[[0, nparts]] + [list(d) for d in bounce.ap])
    nc.gpsimd.dma_start(dst, src)
    return dst


def _bcast2(nc, pools, dram_pool, s_bf, rr):
    """Bounce SBUF [1, 2, rr] (s;t) -> SBUF [128, 2*rr] broadcast."""
    bounce = dram_pool.tile([1, 2 * rr], BF16, tag="bounce_st")
    nc.scalar.dma_start(bounce, s_bf.rearrange("o t r -> o (t r)"))
    dst = pools["bc"].tile([128, 2 * rr], BF16, tag="bc_st", bufs=3)
    src = bass.AP(tensor=bounce.tensor, offset=bounce.offset,
                  ap=[[0, 128]] + [list(d) for d in bounce.ap])
    nc.gpsimd.dma_start(dst, src)
    return dst


def _emit_silu(tc, pools, x_sb, rr):
    nc = tc.nc
    silu = pools["sb"].tile([128, NC_IN, rr], BF16, tag="silu", bufs=3)
    nc.scalar.activation(silu, x_sb, AF.Silu)
    return silu


def _prep_stats(tc, pools, consts, dram_pool, x_sb, rr):
    """LN stats for one tensor -> broadcast s,t tiles [128, 2*rr]."""
    nc = tc.nc
    sb = pools["sb"]
    ones_b = consts["ones128b"]
    xsq = sb.tile([128, NC_IN, rr], BF16, tag="qi")
    nc.vector.tensor_mul(xsq, x_sb, x_sb)
    sums = pools["ps_stat"].tile([1, rr], F32, tag="sums")
    sumsq = pools["ps_stat"].tile([1, rr], F32, tag="sumsq")
    for c in range(NC_IN):
        nc.tensor.matmul(sums, lhsT=ones_b, rhs=x_sb[:, c, :],
                         start=(c == 0), stop=(c == NC_IN - 1))
    for c in range(NC_IN):
        nc.tensor.matmul(sumsq, lhsT=ones_b, rhs=xsq[:, c, :],
                         start=(c == 0), stop=(c == NC_IN - 1))
    stf = sb.tile([1, 2, rr], F32, tag="stats", bufs=1)
    mu, var = stf[:, 0, :], stf[:, 1, :]
    nc.scalar.mul(mu, sums, 1.0 / IN)
    nc.vector.tensor_mul(var, mu, mu)
    # var = sumsq/IN - mu^2 in one fused op
    nc.vector.scalar_tensor_tensor(var, sumsq, 1.0 / IN, var,
                                   OP.mult, OP.subtract)
    # rsqrt via exp(-0.5*ln(var+eps)): stays in the ln/exp table set
    nc.scalar.activation(var, var, AF.Ln, bias=consts["eps"])
    s_bf = sb.tile([1, 2, rr], BF16, tag="sbf", bufs=3)
    nc.scalar.activation(s_bf[:, 0, :], var, AF.Exp, scale=-0.5)
    nc.vector.scalar_tensor_tensor(s_bf[:, 1, :], mu, -1.0, s_bf[:, 0, :],
                                   OP.mult, OP.mult)
    return _bcast2(nc, pools, dram_pool, s_bf, rr)


def _prep_basis(tc, pools, consts, x_sb, silu, st_bc, rr, tt_dve=None):
    """xn + fp8 RBF basis from broadcast stats."""
    nc = tc.nc
    sb = pools["sb"]
    s_bc = st_bc[:, 0:rr]
    t_bc = st_bc[:, rr:2 * rr]
    if tt_dve is None:
        tt_dve = TT_DVE

    # xn = clamp(x*s + t)  (all bf16 SBUF -> DVE 2x)
    xn = sb.tile([128, NC_IN, rr], BF16, tag="xn")
    for c in range(NC_IN):
        nc.vector.tensor_mul(xn[:, c, :], x_sb[:, c, :], s_bc)
    for c in range(NC_IN):
        nc.vector.tensor_add(xn[:, c, :], xn[:, c, :], t_bc)
    nc.vector.tensor_scalar(xn, xn, XCLAMP, -XCLAMP, OP.min, OP.max)

    # anchor t0 = exp(-(1.75*xn + 3.5)^2)
    z0sq = sb.tile([128, NC_IN, rr], F32, tag="z0f", bufs=1)
    nc.scalar.activation(z0sq, xn, AF.Square, scale=1.75, bias=consts["b35"])
    t0 = sb.tile([128, NC_IN, rr], BF16, tag="t0")
    nc.scalar.activation(t0, z0sq, AF.Exp, scale=-1.0)

    # basis_j = t0 * Q_j, Q_j = exp(3.5j*xn + j(7-j)); "A" entries use the
    # exact ACT Square+Exp path, others i16-Schraudolph + TT on DVE/Pool
    basis = sb.tile([128, G, NC_IN, rr], FP8, tag="basis")
    nc.vector.tensor_copy(basis[:, 0], t0)
    for j in range(1, G):
        if tt_dve.get(j) == "A":
            zjsq = sb.tile([128, NC_IN, rr], F32, tag="z0f", bufs=1)
            nc.scalar.activation(zjsq, xn, AF.Square, scale=1.75,
                                 bias=consts["bj"][:, j:j + 1])
            nc.scalar.activation(basis[:, j], zjsq, AF.Exp, scale=-1.0)
            continue
        qi = sb.tile([128, NC_IN, rr], I16, tag="qi")
        nc.vector.tensor_scalar(qi, xn, A16 * 3.5 * j,
                                A16 * j * (7 - j) + B16, OP.mult, OP.add)
        eng = nc.vector if tt_dve.get(j) == "D" else nc.gpsimd
        eng.tensor_tensor(basis[:, j], qi.bitcast(BF16), t0, OP.mult)
    return {"basis": basis, "silu": silu}


def _prep_basis16(tc, pools, consts, x_sb, silu, st_bc, rr):
    """bf16 basis for the lo layer (fp8 noise on the final layer hits the
    output directly; bf16 keeps it at baseline accuracy)."""
    nc = tc.nc
    sb = pools["sb"]
    s_bc = st_bc[:, 0:rr]
    t_bc = st_bc[:, rr:2 * rr]
    xn = sb.tile([128, NC_IN, rr], BF16, tag="xn")
    for c in range(NC_IN):
        nc.vector.tensor_mul(xn[:, c, :], x_sb[:, c, :], s_bc)
    for c in range(NC_IN):
        nc.vector.tensor_add(xn[:, c, :], xn[:, c, :], t_bc)
    nc.vector.tensor_scalar(xn, xn, XCLAMP, -XCLAMP, OP.min, OP.max)
    z0sq = sb.tile([128, NC_IN, rr], F32, tag="z0f", bufs=1)
    nc.scalar.activation(z0sq, xn, AF.Square, scale=1.75, bias=consts["b35"])
    t0 = sb.tile([128, NC_IN, rr], BF16, tag="t0")
    nc.scalar.activation(t0, z0sq, AF.Exp, scale=-1.0)
    slices = [t0]
    for j in range(1, G):
        bj = pools["sb"].tile([128, NC_IN, rr], BF16, tag="A", bufs=8,
                              name=f"b16_{j}")
        if TT_DVE3.get(j) == "A":
            zjsq = sb.tile([128, NC_IN, rr], F32, tag="z0f", bufs=1)
            nc.scalar.activation(zjsq, xn, AF.Square, scale=1.75,
                                 bias=consts["bj"][:, j:j + 1])
            nc.scalar.activation(bj, zjsq, AF.Exp, scale=-1.0)
        else:
            qi = sb.tile([128, NC_IN, rr], I16, tag="qi")
            nc.vector.tensor_scalar(qi, xn, A16 * 3.5 * j,
                                    A16 * j * (7 - j) + B16, OP.mult, OP.add)
            eng = nc.vector if TT_DVE3.get(j) == "D" else nc.gpsimd
            eng.tensor_tensor(bj, qi.bitcast(BF16), t0, OP.mult)
        slices.append(bj)
    return {"slices": slices, "silu": silu}


def _mm_tensor16(tc, pools, io, state, lname, epilogue, rr):
    """bf16 spline + base matmuls (lo layer)."""
    nc = tc.nc
    slices, silu = state["slices"], state["silu"]
    for m in range(NM):
        wt = pools["wt"].tile([128, G, NC_IN, 128], BF16, tag="wt16",
                              bufs=2)
        nc.sync.dma_start(
            wt, io[lname + "_swp16"][:, :, :, 128 * m:128 * (m + 1)]
            .rearrange("j c i m -> i j c m"))
        bwt = pools["wt"].tile([128, NC_IN, 128], BF16, tag="bwt")
        nc.sync.dma_start(
            bwt, io[lname + "_bwp"][:, :, 128 * m:128 * (m + 1)]
            .rearrange("c i m -> i c m"))
        ps = pools["ps_mm"].tile([128, rr], F32, tag="mm")
        rev = (m % 2 == 1)
        korder = [(j, c) for j in range(G) for c in range(NC_IN)]
        if rev:
            korder = korder[::-1]
            for c in range(NC_IN):
                nc.tensor.matmul(ps, lhsT=bwt[:, c], rhs=silu[:, c, :],
                                 start=(c == 0), stop=False)
        for i, (j, c) in enumerate(korder):
            nc.tensor.matmul(ps, lhsT=wt[:, j, c], rhs=slices[j][:, c, :],
                             start=(not rev and i == 0),
                             stop=(rev and i == len(korder) - 1))
        if not rev:
            for c in range(NC_IN):
                nc.tensor.matmul(ps, lhsT=bwt[:, c], rhs=silu[:, c, :],
                                 start=False, stop=(c == NC_IN - 1))
        epilogue(nc, ps, m)


def _mm_tensor(tc, pools, io, state, layers, rr):
    """fp8 DoubleRow spline + bf16 base matmuls per layer / m-tile."""
    nc = tc.nc
    basis, silu = state["basis"], state["silu"]
    for (lname, epilogue) in layers:
        for m in range(NM):
            wt = pools["wt"].tile([128, 2 * G, 2, 128], FP8, tag="wt",
                                  bufs=2)
            nc.sync.dma_start(wt, io[lname + "_swp"][:, :, :,
                                                     128 * m:128 * (m + 1)])
            bwt = pools["wt"].tile([128, NC_IN, 128], BF16, tag="bwt")
            nc.sync.dma_start(
                bwt, io[lname + "_bwp"][:, :, 128 * m:128 * (m + 1)]
                .rearrange("c i m -> i c m"))
            ps = pools["ps_mm"].tile([128, rr], F32, tag="mm")
            rev = (m % 2 == 1)
            if rev:
                for c in range(NC_IN):
                    nc.tensor.matmul(ps, lhsT=bwt[:, c], rhs=silu[:, c, :],
                                     start=(c == 0), stop=False)
            porder = list(range(2 * G - 1, -1, -1)) if rev \
                else list(range(2 * G))
            for i, p in enumerate(porder):
                j, ch = p // 2, p % 2
                nc.tensor.matmul(ps, lhsT=wt[:, p],
                                 rhs=basis[:, j, 2 * ch:2 * ch + 2, :],
                                 perf_mode=PM.DoubleRow,
                                 start=(not rev and i == 0),
                                 stop=(rev and i == 2 * G - 1))
            if not rev:
                for c in range(NC_IN):
                    nc.tensor.matmul(ps, lhsT=bwt[:, c], rhs=silu[:, c, :],
                                     start=False, stop=(c == NC_IN - 1))
            epilogue(nc, ps, m)


def _build_program():
    nc = bacc.Bacc("TRN2", target_bir_lowering=False, debug=False,
                   num_devices=NCORES)
    io = {}
    io["xT3"] = nc.dram_tensor("xT3", [3, IN, R], BF16,
                               kind="ExternalInput").ap()
    for l in LAYERS:
        if l == "lo":
            io["lo_swp16"] = nc.dram_tensor(
                "lo_swp16", [G, NC_IN, 128, OUT], BF16,
                kind="ExternalInput").ap()
        else:
            io[l + "_swp"] = nc.dram_tensor(
                l + "_swp", [128, 2 * G, 2, OUT], FP8,
                kind="ExternalInput").ap()
        io[l + "_bwp"] = nc.dram_tensor(l + "_bwp", [NC_IN, 128, OUT], BF16,
                                        kind="ExternalInput").ap()
        io[l + "_bb"] = nc.dram_tensor(l + "_bb", [NM, 128], F32,
                                       kind="ExternalInput").ap()
    io["outT"] = nc.dram_tensor("outT", [NM, 128, R], F32,
                                kind="ExternalOutput").ap()

    with tile.TileContext(nc) as tc:
        with tc.tile_pool(name="dram", bufs=2, space="DRAM") as dram_pool, \
             tc.tile_pool(name="dram1", bufs=1, space="DRAM") as dram1, \
             tc.tile_pool(name="sb", bufs=2) as sb_pool, \
             tc.tile_pool(name="wt", bufs=3) as wt_pool, \
             tc.tile_pool(name="bc", bufs=2) as bc_pool, \
             tc.tile_pool(name="eo", bufs=2) as eo_pool, \
             tc.tile_pool(name="consts", bufs=1) as cpool:

            # collective buffers
            a2a1a_in = dram1.tile([NCORES, 2, D, R], FP8, tag="a1a_i")
            a2a1a_out = dram1.tile([NCORES, 2, D, R], FP8, tag="a1a_o")
            # ty 0:2 = sg (bf16 [D, R] as [2, D, R/2]); ty 2 = v fp8 bytes
            a2a1b_in = dram1.tile([NCORES, 3, D, R // 2], BF16, tag="a1b_i")
            a2a1b_out = dram1.tile([NCORES, 3, D, R // 2], BF16, tag="a1b_o")
            a2a2_in = dram1.tile([NCORES, D, R], BF16, tag="a2_i")
            a2a2_out = dram1.tile([NCORES, D, R], BF16, tag="a2_o")

            pools = {"sb": sb_pool, "wt": wt_pool, "bc": bc_pool,
                     "eo": eo_pool}

            ones128b = cpool.tile([128, 1], BF16, tag="onesb")
            nc.vector.memset(ones128b, 1.0)
            ones1b = cpool.tile([1, D], BF16, tag="ones1b")
            nc.vector.memset(ones1b, 1.0)
            epst = cpool.tile([1, 1], F32, tag="eps")
            nc.vector.memset(epst, EPS)
            b35 = cpool.tile([128, 1], F32, tag="b35")
            nc.vector.memset(b35, 3.5)
            bj = cpool.tile([128, G], F32, tag="bj")
            for j in range(G):
                nc.vector.memset(bj[:, j:j + 1], 3.5 - j)
            identb = cpool.tile([128, 128], BF16, tag="identb")
            make_identity(nc, identb)
            ident8 = cpool.tile([128, 128], FP8, tag="ident8")
            nc.vector.tensor_copy(ident8, identb)
            consts = {"ones128b": ones128b, "eps": epst, "b35": b35,
                      "bj": bj, "ones1b": ones1b}
            bb = {}
            for l in LAYERS:
                bb[l] = cpool.tile([128, NM], F32, tag=f"bb_{l}",
                                   name=f"bb_{l}")
                nc.sync.dma_start(bb[l], io[l + "_bb"].rearrange("m p -> p m"))

            def epi_qk(ttype, lname):
                def _epi(nc, ps, m):
                    eo = pools["eo"].tile([128, R], FP8, tag="eo8")
                    nc.scalar.activation(eo, ps, AF.Identity,
                                         bias=bb[lname][:, m:m + 1],
                                         scale=1.0 / SCALE)
                    nc.scalar.dma_start(a2a1a_in[2 * m, ttype], eo[0:D, :])
                    nc.scalar.dma_start(a2a1a_in[2 * m + 1, ttype],
                                        eo[D:2 * D, :])
                return _epi

            def epi_g(nc, ps, m):
                eo = pools["eo"].tile([128, R], BF16, tag="eo2", bufs=1)
                nc.scalar.activation(eo, ps, AF.Sigmoid,
                                     bias=bb["lg"][:, m:m + 1],
                                     scale=1.0 / SCALE)
                for i in range(2):
                    nc.scalar.dma_start(
                        a2a1b_in[2 * m + i, 0:2].rearrange("t p n -> p t n"),
                        eo[D * i:D * (i + 1), :]
                        .rearrange("p (t n) -> p t n", t=2))

            def epi_v(nc, ps, m):
                eo = pools["eo"].tile([128, R], FP8, tag="eo8")
                nc.scalar.activation(eo, ps, AF.Identity,
                                     bias=bb["lv"][:, m:m + 1],
                                     scale=1.0 / SCALE)
                for i in range(2):
                    nc.scalar.dma_start(a2a1b_in[2 * m + i, 2],
                                        eo[D * i:D * (i + 1), :].bitcast(BF16))

            def epi_out_ch(ch, rr):

                def _epi(nc, ps, m):
                    eo = pools["eo"].tile([128, rr], F32, tag="eo4", bufs=1)
                    nc.scalar.activation(eo, ps, AF.Identity,
                                         bias=bb["lo"][:, m:m + 1],
                                         scale=1.0 / SCALE)
                    nc.scalar.dma_start(io["outT"][m][:, rr * ch:rr * (ch + 1)],
                                        eo)
                return _epi

            def load_xT(idx):
                x = pools["sb"].tile([128, NC_IN, R], BF16, tag="xT", bufs=3)
                eng = [nc.sync, nc.scalar, nc.gpsimd][idx]
                eng.dma_start(
                    x, io["xT3"][idx].rearrange("(c p) r -> p c r", p=128))
                return x

            rg = [list(range(NCORES))]
            nocc = bool(int(os.environ.get("KERNEL_NOCC", "0")))

            def a2a(in_ap, out_ap):
                if nocc:
                    nc.sync.dma_start(out_ap, in_ap)
                else:
                    nc.gpsimd.collective_compute(
                        "AllToAll", OP.bypass, replica_groups=rg,
                        ins=[in_ap.opt()], outs=[out_ap.opt()])

            A_tiles = {}

            with tc.tile_pool(name="ps_S", bufs=2, space="PSUM") as ps_S:

                def emit_S_exp(qc, wq_b, wk_b, kts=None):
                    q0 = QC * qc
                    for kt in (kts if kts is not None else range(NKT)):
                        if kt % 2 == 0:
                            A_tiles[(qc, kt // 2)] = pools["sb"].tile(
                                [128, 2, B, QC], FP8, tag="A", bufs=8,
                                name=f"A_{qc}_{kt // 2}")
                        A_t = A_tiles[(qc, kt // 2)]
                        for b in range(B):
                            S_ps = ps_S.tile([128, QC], F32, tag="S")
                            for h2 in range(QC // 512):
                                nc.tensor.matmul(
                                    S_ps[:, 512 * h2:512 * (h2 + 1)],
                                    lhsT=wk_b[:, b, :,
                                              128 * kt:128 * (kt + 1)],
                                    rhs=wq_b[:, b, :,
                                             q0 + 512 * h2:
                                             q0 + 512 * (h2 + 1)],
                                    perf_mode=PM.DoubleRow,
                                    start=True, stop=True)
                            unit = qc * 32 + kt * 2 + b
                            lim = EXPDVE0 if qc == 0 else EXPDVE1
                            if unit % 16 < lim:
                                nc.vector.tensor_scalar(
                                    A_t[:, kt % 2, b, :].bitcast(
                                        mybir.dt.int8),
                                    S_ps, A8S, B8S, OP.mult, OP.add)
                            else:
                                nc.scalar.activation(
                                    A_t[:, kt % 2, b, :], S_ps, AF.Exp)

                # ------------------------------------------------- phase 1
                with tc.tile_pool(name="ps_mm", bufs=2,
                                  space="PSUM") as ps_mm, \
                     tc.tile_pool(name="ps_stat", bufs=1,
                                  space="PSUM") as ps_stat:
                    pools["ps_mm"] = ps_mm
                    pools["ps_stat"] = ps_stat

                    x_k = load_xT(1)
                    x_q = load_xT(0)
                    x_v = load_xT(2)
                    silu_k = _emit_silu(tc, pools, x_k, R)
                    silu_q = _emit_silu(tc, pools, x_q, R)
                    silu_v = _emit_silu(tc, pools, x_v, R)

                    # all three LN-stats first: keeps the Ln/Exp table
                    # resident (one load) and fires the bounces early
                    bc_k = _prep_stats(tc, pools, consts, dram_pool, x_k, R)
                    bc_q = _prep_stats(tc, pools, consts, dram_pool, x_q, R)
                    bc_v = _prep_stats(tc, pools, consts, dram_pool, x_v, R)
                    st_k = _prep_basis(tc, pools, consts, x_k, silu_k,
                                       bc_k, R)
                    st_q = _prep_basis(tc, pools, consts, x_q, silu_q,
                                       bc_q, R)
                    _mm_tensor(tc, pools, io, st_k,
                               [("lk", epi_qk(1, "lk"))], R)
                    _mm_tensor(tc, pools, io, st_q,
                               [("lq", epi_qk(0, "lq"))], R)
                    a2a(a2a1a_in, a2a1a_out)

                    _mm_tensor(tc, pools, io, st_q, [("lg", epi_g)], R)
                    st_v = _prep_basis(tc, pools, consts, x_v, silu_v,
                                       bc_v, R)

                    # phase-2 early: wq/wk loads + S/exp for qc0, interleaved
                    # with lv so its epilogues aren't stuck behind exp in the
                    # ACT FIFO
                    wq_b = sb_pool.tile([32, B, 2, L], FP8, tag="wq", bufs=1)
                    wk_b = sb_pool.tile([32, B, 2, L], FP8, tag="wk", bufs=1)
                    for b in range(B):
                        for (dst, ty) in ((wq_b, 0), (wk_b, 1)):
                            for r in range(4):
                                nc.scalar.dma_start(
                                    dst[:, b, :, 512 * r:512 * (r + 1)],
                                    a2a1a_out[4 * b + r, ty]
                                    .rearrange("(h p) n -> p h n", h=2))
                    emit_S_exp(0, wq_b, wk_b, range(0, 8))
                    _mm_tensor(tc, pools, io, st_v, [("lv", epi_v)], R)
                    emit_S_exp(0, wq_b, wk_b, range(8, 16))
                    a2a(a2a1b_in, a2a1b_out)

                # ------------------------------------------------- phase 2
                wvT_b = sb_pool.tile([128, L], FP8, tag="wvT", bufs=1)
                sg_t = sb_pool.tile([D, B, L], BF16, tag="sg", bufs=1)
                for b in range(B):
                    for r in range(4):
                        nc.sync.dma_start(
                            wvT_b[D * b:D * (b + 1),
                                  512 * r:512 * (r + 1)],
                            a2a1b_out[4 * b + r, 2].bitcast(FP8))
                    for r in range(4):
                        nc.sync.dma_start(
                            sg_t[:, b, 512 * r:512 * (r + 1)]
                            .rearrange("p (t n) -> p t n", t=2),
                            a2a1b_out[4 * b + r, 0:2]
                            .rearrange("t p n -> p t n"))

                # m padded to 128 (DoubleRow needs 64-aligned stationary
                # free); rows 65-127 of attv are garbage and never read
                wv_aug = sb_pool.tile([128, B, G, 2, 2 * D], FP8,
                                      tag="wvaug", bufs=1)
                onesw = cpool.tile([128, 32], BF16, tag="onesw")
                nc.vector.memset(onesw, 1.0)
                nc.vector.tensor_copy(
                    wv_aug[:, :, :, :, D],
                    onesw.rearrange("p (b g t) -> p b g t", b=B, g=G))
                for b in range(B):
                    for kt4 in range(NKT // 4):
                        tp = ps_S.tile([128, QC], F32, tag="S")
                        # fp8 transpose requires output element step 2
                        tp8 = tp.bitcast(FP8)[:, 0:4 * D * 2].rearrange(
                            "p (k d s) -> p k d s", d=D, s=2)
                        for i in range(4):
                            kt = kt4 * 4 + i
                            nc.tensor.transpose(
                                tp8[:, i, :, 0],
                                wvT_b[D * b:D * (b + 1),
                                      128 * kt:128 * (kt + 1)],
                                ident8[D * b:D * (b + 1),
                                       D * b:D * (b + 1)])
                        nc.vector.tensor_copy(
                            wv_aug[:, b, 2 * kt4:2 * kt4 + 2, :, 0:D]
                            .rearrange("p g t d -> p (g t) d"),
                            tp8[:, :, :, 0])

                with tc.tile_pool(name="ps_av", bufs=1, space="PSUM") as ps_av:
                    def emit_av_epi(qc):
                        q0 = QC * qc
                        attv = [ps_av.tile([2 * D, QC], F32, tag=f"attv{b}",
                                           name=f"attv{b}_{qc}")
                                for b in range(B)]
                        for pr in range(G):
                            A_t = A_tiles[(qc, pr)]
                            for b in range(B):
                                for h2 in range(QC // 512):
                                    nc.tensor.matmul(
                                        attv[b][:, 512 * h2:512 * (h2 + 1)],
                                        lhsT=wv_aug[:, b, pr],
                                        rhs=A_t[:, :, b,
                                                512 * h2:512 * (h2 + 1)],
                                        perf_mode=PM.DoubleRow,
                                        start=(pr == 0), stop=(pr == G - 1))
                        for b in range(B):
                            recip = pools["sb"].tile([1, QC], BF16,
                                                     tag="rcp", bufs=2)
                            with nc.allow_low_precision(
                                    reason="softmax denom recip in bf16"):
                                nc.vector.reciprocal(recip,
                                                     attv[b][D:D + 1, :])
                            rb = _bcast(nc, pools, dram_pool, recip, QC, D,
                                        f"r{b}")
                            t1 = pools["sb"].tile([D, QC], BF16, tag="t1",
                                                  bufs=1)
                            nc.vector.tensor_mul(t1, attv[b][0:D, :], rb)
                            og = t1
                            nc.vector.tensor_mul(
                                og, t1, sg_t[:, b, q0:q0 + QC])
                            nc.sync.dma_start(a2a2_in[4 * b + 2 * qc],
                                              og[:, 0:512])
                            nc.sync.dma_start(a2a2_in[4 * b + 2 * qc + 1],
                                              og[:, 512:QC])

                    emit_av_epi(0)
                    emit_S_exp(1, wq_b, wk_b)
                    emit_av_epi(1)
                    a2a(a2a2_in, a2a2_out)

            # ----------------------------------------------------- phase 3
            with tc.tile_pool(name="ps_mm3", bufs=2, space="PSUM") as ps_mm3, \
                 tc.tile_pool(name="ps_stat3", bufs=1,
                              space="PSUM") as ps_stat3:
                pools["ps_mm"] = ps_mm3
                pools["ps_stat"] = ps_stat3
                x3 = pools["sb"].tile([128, NC_IN, R], BF16, tag="xT",
                                      bufs=3, name="x3")
                nc.sync.dma_start(
                    x3, a2a2_out.rearrange("(c h) d n -> (h d) c n", c=4))
                silu_o = _emit_silu(tc, pools, x3, R)
                bc_o = _prep_stats(tc, pools, consts, dram_pool, x3, R)
                st_o = _prep_basis16(tc, pools, consts, x3, silu_o, bc_o, R)
                _mm_tensor16(tc, pools, io, st_o, "lo", epi_out_ch(0, R), R)

    nc.compile()
    return nc


# ------------------------------------------------------------------------- host
def _prep_layer(inputs, name, scale=1.0):
    if name == "lo":
        sw = np.asarray(inputs[name + "_sw"], np.float32) * (scale * SCALE)
        bw = np.asarray(inputs[name + "_bw"], np.float32) * (scale * SCALE)
        bbv = np.asarray(inputs[name + "_bb"], np.float32) * scale
        swp16 = sw.reshape(OUT, NC_IN, 128, G).transpose(3, 1, 2, 0)
        return {"lo_swp16": np.ascontiguousarray(swp16.astype(BF16NP)),
                "lo_bwp": np.ascontiguousarray(
                    bw.T.reshape(NC_IN, 128, OUT).astype(BF16NP)),
                "lo_bb": np.ascontiguousarray(bbv.reshape(NM, 128))}
    sw = np.asarray(inputs[name + "_sw"], np.float32) * (scale * SCALE)
    bw = np.asarray(inputs[name + "_bw"], np.float32) * (scale * SCALE)
    bbv = np.asarray(inputs[name + "_bb"], np.float32) * scale
    assert np.all(np.asarray(inputs[name + "_ln_s"]) == 1.0)
    assert np.all(np.asarray(inputs[name + "_ln_b"]) == 0.0)
    # sw [OUT, IN*G], flat = i*G + j, i = c*128 + p, c = ch*2 + c01
    # -> [p, j, ch, c01, out] -> [p, pair=(j,ch), c01, out]
    swp = sw.reshape(OUT, 2, 2, 128, G).transpose(3, 4, 1, 2, 0)
    swp = np.clip(swp, -240.0, 240.0).astype(E4M3)
    swp = swp.reshape(128, 2 * G, 2, OUT)
    bwp = bw.T.reshape(NC_IN, 128, OUT).astype(BF16NP)
    return {name + "_swp": np.ascontiguousarray(swp),
            name + "_bwp": np.ascontiguousarray(bwp),
            name + "_bb": np.ascontiguousarray(bbv.reshape(NM, 128))}


def kernel(**inputs):
    if "nc" not in _cache:
        _cache["nc"] = _build_program()
    nc = _cache["nc"]

    norm = float(D) ** -0.5
    w = {}
    for l, sc in (("lq", norm), ("lg", 1.0), ("lk", 1.0), ("lv", 1.0),
                  ("lo", 1.0)):
        w.update(_prep_layer(inputs, l, sc))

    q = np.asarray(inputs["q"], np.float32).reshape(B * L, IN)
    k = np.asarray(inputs["k"], np.float32).reshape(B * L, IN)
    v = np.asarray(inputs["v"], np.float32).reshape(B * L, IN)

    in_maps = []
    for core in range(NCORES):
        rows = slice(R * core, R * (core + 1))
        xT3 = np.stack([np.ascontiguousarray(q[rows].T),
                        np.ascontiguousarray(k[rows].T),
                        np.ascontiguousarray(v[rows].T)]).astype(BF16NP)
        m = {"xT3": xT3}
        m.update(w)
        in_maps.append(m)

    res = run_bass_kernel_spmd(nc, in_maps, core_ids=list(range(NCORES)))
    _cache["last_result"] = res

    out = np.zeros((B * L, OUT), np.float32)
    for core in range(NCORES):
        rows = slice(R * core, R * (core + 1))
        out[rows, :] = res.results[core]["outT"].reshape(OUT, R).T
    return out.reshape(B, L, OUT)
